# revision 37
# baseline (speedup 1.0000x reference)
"""Trainium2 Bass kernel for a binarized (1w/1a) BasicBlock — fp8 DoubleRow.

    a1 = sign(x);  y1 = BN(conv3x3(a1, binarize(w1))) + x;  x1 = maxout(y1)
    a2 = sign(x1); y2 = BN(conv3x3(a2, binarize(w2))) + x1; out = maxout(y2)

Data-parallel over batch (4 samples/core, 8 cores); exact binary math:
activations are +-1 (fp8e4, exact), weights are sign(+-1) fp8; each conv is
9 DoubleRow matmuls per (chunk, cout-block), contracting all 256 input
channels at once over contiguous padded-row runs (pad columns land in
unused psum columns).  The two channel-block planes are byte-interleaved
per pixel so each DoubleRow rhs column is one 2-byte fetch (split planes
measured ~1.9 cycles/column; interleaved targets ~1).  conv_true = alpha_a*alpha[o]*(BB + q[o]*S1) with
q = beta/alpha; S1 (3x3 box of the channel sum) comes from 3 more DoubleRow
ones-matmuls (folding the kh taps) + 2 shifted adds.  The per-channel scale
folds into BN exactly by scaling BN_EPS per channel.

Maxout exploits pos==1: out = max(t, neg*t) — one DVE/gp STT, no sign/coef.

Batch-stat BN: one AllGather of (sum, sumsq) [128,4] per round, with the
cross-rank add done locally on DVE.  The ncfw collective stack costs
~30us of cold init (self-started from the NRT preamble ~11us in) plus
~25us on the first payload op of every execution, so a warmup AllGather
is triggered first-thing to absorb both under conv1; the two real AGs
then run at the ~5-10us steady-state floor.  Warm-up and gated dummy
matmuls bridge the PE across the stats gaps so HAM keeps the 2.4 GHz
clock.  (A GpSimd remote_dma_broadcast XOR-slot exchange would cut the
stats latency to ~3us and drop ncfw entirely, but the Tile scheduler's
single-core simulation cannot model cross-core semaphore arrivals and
deadlocks on the required wait_ge — revisit if that gains support.)
"""

import numpy as np
import ml_dtypes

import concourse.bass as bass
import concourse.bacc as bacc
import concourse.mybir as mybir
import concourse.tile as tile

N_CORES = 8
B, C, H, W = 32, 256, 28, 28
BPC = B // N_CORES            # samples per core
NBLK = 2                      # channel blocks of 128
HPAD, WPAD = 30, 32           # padded image in SBUF
PIX = H * W                   # 784
PPIX = HPAD * WPAD            # 960
NCHUNK = 2 * BPC              # 8 chunks of (sample, half-image)
HHALF = H // 2                # 14
CHUNK = HHALF * W             # 392 dense output elems per chunk
RUN = HHALF * WPAD            # 448: rhs run length / psum width per chunk
BN_EPS = 1e-5
NPRM = 20
GUARD = 32                    # fp8 guard elems around merged activation tile
PLANE = BPC * PPIX            # 3840 elems per channel-block plane
SPAN = 4 * CHUNK              # 1568-wide apply spans (2 samples)
CNT = float(NCHUNK * CHUNK * N_CORES)   # 25088 elems per channel for BN
F32 = mybir.dt.float32
BF16 = mybir.dt.bfloat16
FP8 = mybir.dt.float8e4
AF = mybir.ActivationFunctionType
ALU = mybir.AluOpType
DR = mybir.MatmulPerfMode.DoubleRow
RG = [list(range(N_CORES))]


class Rnd:
    """One conv+BN+residual+maxout round's issue helpers."""

    def __init__(self, nc, tc, pools, rnd, apad, wsb, cv, prm, pcol, onesb,
                 out_d=None, a2pad=None, xres=None, x1b=None):
        self.__dict__.update(locals())
        sbuf, psum, sc, dram = pools
        self.sc, self.psum, self.sbuf, self.dram = sc, psum, sbuf, dram
        self.sums = [sbuf.tile([128, NCHUNK], F32, name=f"sums{rnd}_{i}")
                     for i in range(NBLK)]
        self.sumsqs = [sbuf.tile([128, NCHUNK], F32, name=f"sumsqs{rnd}_{i}")
                       for i in range(NBLK)]
        # sample-major activation layout (b, i, h, w): the rhs AP's
        # contiguous dependency footprint stays inside one sample
        self.t448 = apad[:, 0:2 * PPIX].rearrange(
            "p (n i) -> p i n", i=2)[:, :, 0:RUN]
        self.wv = wsb[:].rearrange("p (k i o) -> p k i o", k=9, i=NBLK)
        self.ones3 = onesb[:].rearrange("p (i o) -> p i o", i=2)
        self.s1s = {}
        self.ss = {}
        self.tots = {}
        self.gst_dmas = {}
        self.tts = {}
        if a2pad is not None:
            self.a2v = a2pad[:, GUARD:GUARD + 2 * PLANE].rearrange(
                "p (b h w i) -> p b i h w", b=BPC, i=2, h=HPAD, w=WPAD)
        if out_d is not None:
            self.ov = out_d[:].rearrange("(bp b2) c h w -> c bp b2 (h w)",
                                         bp=2)

    def rhs_ap(self, b, off):
        return bass.AP(self.t448.tensor, GUARD + b * 2 * PPIX + 2 * off,
                       self.t448.ap)

    def s1_build(self, b):
        """S1(b) = 3x3 box of the channel sum: 3 kh-folded ones-matmuls per
        half (PE) -> hs evac (ACT) -> w-taps on gp + DVE."""
        nc, sc, psum, rnd = self.nc, self.sc, self.psum, self.rnd
        hs = sc.tile([128, 2 * RUN], F32, tag="hs", name="hs", bufs=2)
        for half in range(2):
            h0 = half * HHALF
            ps2 = psum.tile([128, RUN], F32, tag="ps2", name=f"ps2_{rnd}",
                            bufs=2)
            for kh in range(3):
                nc.tensor.matmul(ps2[:], self.ones3,
                                 self.rhs_ap(b, (h0 + kh) * WPAD),
                                 start=(kh == 0), stop=(kh == 2),
                                 perf_mode=DR)
            nc.scalar.copy(hs[:, half * RUN:half * RUN + RUN], ps2[:])
        hsv = hs[:].rearrange("p (h w) -> p h w", h=H)
        w3 = sc.tile([128, H * W], F32, tag="w3", name="w3", bufs=2)
        w3v = w3[:].rearrange("p (h w) -> p h w", h=H)
        # always DVE: GpSimd tensor ops contend with the DVE for the one
        # shared SBUF port (measured 680ns DVE TS ops degrade to 2.2us
        # whenever a gp tensor_add runs concurrently)
        nc.vector.tensor_add(w3v, hsv[:, :, 1:1 + W], hsv[:, :, 2:2 + W])
        s1 = sc.tile([128, H * W], F32, tag="s1", name="s1", bufs=BPC)
        s1v = s1[:].rearrange("p (h w) -> p h w", h=H)
        nc.vector.tensor_add(s1v, w3v, hsv[:, :, 3:3 + W])
        self.s1s[b] = s1

    def chunk(self, ci, oblk):
        """9 DoubleRow matmuls + evac: cv = q*S1 + BB (DVE STT, accumulates
        sum) and a Square pass on ScalarE (accumulates sumsq)."""
        nc, rnd = self.nc, self.rnd
        b, h0 = ci // 2, (ci % 2) * HHALF
        ps = self.psum.tile([128, RUN], F32, tag="ps", name=f"ps{rnd}",
                            bufs=6)
        for k9 in range(9):
            kh, kw = k9 // 3, k9 % 3
            nc.tensor.matmul(
                ps[:], self.wv[:, k9, :, oblk * 128:(oblk + 1) * 128],
                self.rhs_ap(b, (h0 + kh) * WPAD + kw - 1),
                start=(k9 == 0), stop=(k9 == 8), perf_mode=DR)
        psv = ps[:].rearrange("p (h w) -> p h w", h=HHALF)[:, :, 2:2 + W]
        s1v = self.s1s[b][:].rearrange("p (h w) -> p h w", h=H)[
            :, (ci % 2) * HHALF:(ci % 2) * HHALF + HHALF, :]
        cvc = self.cv[oblk][:, ci * CHUNK:(ci + 1) * CHUNK]
        nc.vector.scalar_tensor_tensor(
            cvc.rearrange("p (h w) -> p h w", h=HHALF), s1v,
            self.prm[:, self.pcol['q'] + oblk:self.pcol['q'] + oblk + 1],
            psv, op0=ALU.mult, op1=ALU.add,
            accum_out=self.sums[oblk][:, ci:ci + 1])
        sqj = self.sc.tile([128, CHUNK], F32, tag="sqj", name="sqj", bufs=2)
        nc.scalar.activation(
            sqj[:], cvc, AF.Square,
            accum_out=self.sumsqs[oblk][:, ci:ci + 1])

    def loc_reduce(self, oblk, loc4):
        """Reduce this block's per-chunk stats into loc4 cols [2b, 2b+2);
        returns the two reduce instructions (deps for the trigger)."""
        nc = self.nc
        r0 = nc.vector.reduce_sum(loc4[:, 2 * oblk:2 * oblk + 1],
                                  self.sums[oblk][:],
                                  axis=mybir.AxisListType.X)
        r1 = nc.vector.reduce_sum(loc4[:, 2 * oblk + 1:2 * oblk + 2],
                                  self.sumsqs[oblk][:],
                                  axis=mybir.AxisListType.X)
        return [r0, r1]

    def fin2(self, tot4):
        """Global (sum, sumsq) -> BN scale/shift for BOTH blocks in one
        [128,2]-wide op chain (halves the fin's cross-engine ping-pong)."""
        nc, rnd, prm, pcol = self.nc, self.rnd, self.prm, self.pcol
        sbuf = self.sbuf
        tv = tot4.rearrange("p (b j) -> p j b", b=2)
        mm = sbuf.tile([128, 2], F32, name=f"mm{rnd}")
        mq = sbuf.tile([128, 2], F32, name=f"mq{rnd}")
        var = sbuf.tile([128, 2], F32, name=f"var{rnd}")
        sd = sbuf.tile([128, 2], F32, name=f"sd{rnd}")
        inv = sbuf.tile([128, 2], F32, name=f"inv{rnd}")
        scale = sbuf.tile([128, 2], F32, name=f"scale{rnd}")
        tmp = sbuf.tile([128, 2], F32, name=f"tmp{rnd}")
        shift = sbuf.tile([128, 2], F32, name=f"shift{rnd}")
        nc.vector.tensor_scalar_mul(mm[:], tv[:, 0], 1.0 / CNT)
        nc.vector.tensor_scalar_mul(mq[:], tv[:, 1], 1.0 / CNT)
        nc.vector.tensor_mul(tmp[:], mm[:], mm[:])
        nc.vector.tensor_sub(var[:], mq[:], tmp[:])
        nc.vector.tensor_add(var[:], var[:],
                             prm[:, pcol['eps']:pcol['eps'] + 2])
        nc.scalar.activation(sd[:], var[:], AF.Sqrt)
        nc.vector.reciprocal(inv[:], sd[:])
        nc.vector.tensor_mul(scale[:], inv[:],
                             prm[:, pcol['g']:pcol['g'] + 2])
        nc.vector.tensor_mul(tmp[:], mm[:], scale[:])
        nc.vector.tensor_sub(shift[:],
                             prm[:, pcol['b']:pcol['b'] + 2], tmp[:])
        for oblk in range(NBLK):
            self.ss[oblk] = (scale[:, oblk:oblk + 1],
                             shift[:, oblk:oblk + 1])

    # ---- round-1 apply: t = scale*cv + shift + x (f32, exact signs);
    #      a2 = sign(t) (ACT); x1 = max(t, n1*t) deferred on DVE.
    def apply_span1(self, b, oblk, ueng, teng):
        nc, sc, prm, pcol = self.nc, self.sc, self.prm, self.pcol
        scale, shift = self.ss[oblk]
        lo = b * PIX
        cvs = self.cv[oblk][:, lo:lo + PIX]
        uq = nc.gpsimd if ueng == 'gp' else nc.vector
        tq = nc.gpsimd if teng == 'gp' else nc.vector
        u = sc.tile([128, PIX], F32, tag="u", name="u", bufs=3)
        uq.tensor_scalar(u[:], cvs, scale, shift,
                         op0=ALU.mult, op1=ALU.add)
        t = sc.tile([128, PIX], F32, tag="t", name="t", bufs=8)
        tq.tensor_add(t[:], u[:], self.xres[oblk][:, lo:lo + PIX])
        sg = self.a2v[:, b, oblk, 1:1 + H, 2:2 + W]
        nc.scalar.activation(sg, t[:].rearrange("p (h w) -> p h w", h=H),
                             AF.Sign)
        self.tts[(b, oblk)] = t

    def x1_stt(self, b, oblk):
        nc, prm, pcol = self.nc, self.prm, self.pcol
        t = self.tts[(b, oblk)]
        lo = b * PIX
        nc.vector.scalar_tensor_tensor(
            self.x1b[oblk][:, lo:lo + PIX], t[:],
            prm[:, pcol['n'] + oblk:pcol['n'] + oblk + 1], t[:],
            op0=ALU.mult, op1=ALU.max)

    # ---- round-2 apply: u2 = scale*cv + shift (ACT affine),
    #      t2 = u2 + x1, och = max(t2, n2*t2), DMA out.
    def apply_span2(self, sp, oblk, tq, oq, ochq=None):
        nc, sc, prm, pcol = self.nc, self.sc, self.prm, self.pcol
        scale, shift = self.ss[oblk]
        lo = sp * SPAN
        cvs = self.cv[oblk][:, lo:lo + SPAN]
        u2 = sc.tile([128, SPAN], BF16, tag="u2", name="u2", bufs=3)
        nc.scalar.activation(u2[:], cvs, AF.Identity,
                             bias=shift, scale=scale)
        t2 = sc.tile([128, SPAN], BF16, tag="t2", name="t2", bufs=3)
        tq.tensor_add(t2[:], u2[:], self.x1b[oblk][:, lo:lo + SPAN])
        nt = sc.tile([128, SPAN], BF16, tag="nt", name="nt", bufs=3)
        nc.vector.tensor_scalar_mul(
            nt[:], t2[:], prm[:, pcol['n'] + oblk:pcol['n'] + oblk + 1])
        och = sc.tile([128, SPAN], BF16, tag="och", name="och", bufs=3)
        nc.vector.tensor_max(och[:], t2[:], nt[:])
        oq.dma_start(self.ov[oblk * 128:oblk * 128 + 128, sp],
                     och[:].rearrange("p (b2 hw) -> p b2 hw", b2=2))


def build():
    nc = bacc.Bacc("TRN2", target_bir_lowering=False, debug=False,
                   enable_asserts=True, num_devices=N_CORES)
    x_d = nc.dram_tensor("x", [BPC, C, H, W], F32, kind="ExternalInput")
    w1_d = nc.dram_tensor("w1t", [9, NBLK, 128, 256], FP8,
                          kind="ExternalInput")
    w2_d = nc.dram_tensor("w2t", [9, NBLK, 128, 256], FP8,
                          kind="ExternalInput")
    prm_d = nc.dram_tensor("prm", [128, NPRM], F32, kind="ExternalInput")
    out_d = nc.dram_tensor("out", [BPC, C, H, W], BF16,
                           kind="ExternalOutput")

    with tile.TileContext(nc) as tc:
        with (
            tc.tile_pool(name="sbuf", bufs=1) as sbuf,
            tc.tile_pool(name="psum", bufs=6, space="PSUM") as psum,
            tc.tile_pool(name="sc", bufs=2) as sc,
            tc.tile_pool(name="dram", bufs=1, space="DRAM") as dram,
        ):
            pools = (sbuf, psum, sc, dram)
            w1sb = sbuf.tile([128, 9 * NBLK * 256], FP8, name="w1sb")
            w2sb = sbuf.tile([128, 9 * NBLK * 256], FP8, name="w2sb")
            prm = sbuf.tile([128, NPRM], F32, name="prm")
            onesb = sbuf.tile([128, 256], FP8, name="onesb")
            nc.vector.memset(onesb[:], 1.0)
            xres = [sbuf.tile([128, BPC * PIX], F32, name=f"xres{i}")
                    for i in range(NBLK)]
            x1b = [sbuf.tile([128, BPC * PIX], BF16, name=f"x1b{i}")
                   for i in range(NBLK)]
            a1p = sbuf.tile([128, GUARD + 2 * PLANE + GUARD], FP8, name="a1p")
            a2p = sbuf.tile([128, GUARD + 2 * PLANE + GUARD], FP8, name="a2p")
            cv = [sbuf.tile([128, BPC * PIX], F32, name=f"cv{i}")
                  for i in range(NBLK)]
            cvb = [sbuf.tile([128, BPC * PIX], BF16, name=f"cvb{i}")
                   for i in range(NBLK)]

            nc.vector.memset(a1p[:].bitcast(mybir.dt.uint32), 0)
            nc.gpsimd.memset(a2p[:].bitcast(mybir.dt.uint32), 0)
            # x planes all on one ring (sample-major: one ring fans out over
            # all 16 DMA engines at full BW; splitting rings fragments it)
            xv = x_d[:].rearrange("b c h w -> c b (h w)")
            for b in range(BPC):
                for i in range(NBLK):
                    nc.sync.dma_start(xres[i][:, b * PIX:(b + 1) * PIX],
                                      xv[i * 128:(i + 1) * 128, b])
            w1v = w1sb[:].rearrange("p (k i o) -> p k i o", k=9, i=NBLK)
            w1dv = w1_d[:].rearrange("k i p o -> p k i o")
            nc.scalar.dma_start(w1v[:, 0:3], w1dv[:, 0:3])
            nc.gpsimd.dma_start(w1v[:, 3:6], w1dv[:, 3:6])
            nc.sync.dma_start(w1v[:, 6:9], w1dv[:, 6:9])
            nc.sync.dma_start(prm[:], prm_d[:])
            nc.sync.dma_start(
                w2sb[:].rearrange("p (k i o) -> p k i o", k=9, i=NBLK),
                w2_d[:].rearrange("k i p o -> p k i o"))

            pcol1 = {'g': 0, 'b': 2, 'n': 4, 'eps': 12, 'q': 16}
            pcol2 = {'g': 6, 'b': 8, 'n': 10, 'eps': 14, 'q': 18}
            r1 = Rnd(nc, tc, pools, 1, a1p, w1sb, cv, prm, pcol1, onesb,
                     a2pad=a2p, xres=xres, x1b=x1b)
            r2 = Rnd(nc, tc, pools, 2, a2p, w2sb, cvb, prm, pcol2, onesb,
                     out_d=out_d, x1b=x1b)

            # --- stats exchange state (ncfw AllGather; the warmup AG
            # below absorbs the ~30us ncfw init + ~25us first-op cost) ---
            loc41 = sbuf.tile([128, 4], F32, name="loc41")
            loc42 = sbuf.tile([128, 4], F32, name="loc42")
            tot4s = {k: sbuf.tile([128, 4], F32, name=f"tot4_{k}")
                     for k in ("1", "2")}
            # force the gpsimd tensor lib to load in the idle prologue (its
            # lazy ~6us IRAM load otherwise stalls the first transition add)
            gwarm = sbuf.tile([128, 2], F32, name="gwarm")
            nc.vector.memset(gwarm[:], 0.0)
            nc.gpsimd.tensor_add(gwarm[:, 0:1], gwarm[:, 0:1], gwarm[:, 1:2])
            # warmup collective: absorbs the ~30us ncfw init + ~25us
            # first-payload-op cost under conv1
            wu_i = dram.tile([1, 16], F32, name="wu_i")
            wu_o = dram.tile([N_CORES, 16], F32, name="wu_o")
            with tc.high_priority():
                nc.sync.dma_start(
                    wu_i[:],
                    x_d[:].rearrange("b c h w -> (b c h) w")[0:1, 0:16])
                nc.gpsimd.collective_compute(
                    "AllGather", ALU.bypass, replica_groups=RG,
                    ins=[wu_i.opt()], outs=[wu_o.opt()])

            a_ins = {k: dram.tile([128, 4], F32, name=f"agin{k}")
                     for k in ("1", "2")}

            def stage_half(key, loc4, blk):
                nc.sync.dma_start(a_ins[key][:, 2 * blk:2 * blk + 2],
                                  loc4[:, 2 * blk:2 * blk + 2])

            def fire4(loc4, key):
                a_in = a_ins[key]
                a_out = dram.tile([N_CORES, 128, 4], F32, name=f"agout{key}")
                nc.gpsimd.collective_compute(
                    "AllGather", ALU.bypass, replica_groups=RG,
                    ins=[a_in.opt()], outs=[a_out.opt()])
                gst = sbuf.tile([128, N_CORES * 4], F32, name=f"gst{key}")
                gd = nc.sync.dma_start(
                    gst[:].rearrange("p (r j) -> p r j", r=N_CORES),
                    a_out[:].rearrange("r p j -> p r j"))
                nc.vector.reduce_sum(
                    tot4s[key][:],
                    gst[:].rearrange("p (r j) -> p j r", r=N_CORES),
                    axis=mybir.AxisListType.X)
                return gd

            def fire(loc2, key):
                a_in = dram.tile([128, 2], F32, name=f"agin{key}")
                a_out = dram.tile([N_CORES, 128, 2], F32, name=f"agout{key}")
                nc.sync.dma_start(a_in[:], loc2)
                nc.gpsimd.collective_compute(
                    "AllGather", ALU.bypass, replica_groups=RG,
                    ins=[a_in.opt()], outs=[a_out.opt()])
                gst = sbuf.tile([128, N_CORES * 2], F32, name=f"gst{key}")
                gd = nc.sync.dma_start(
                    gst[:].rearrange("p (r j) -> p r j", r=N_CORES),
                    a_out[:].rearrange("r p j -> p r j"))
                nc.vector.reduce_sum(
                    tots[key][:],
                    gst[:].rearrange("p (r j) -> p j r", r=N_CORES),
                    axis=mybir.AxisListType.X)
                return gd

            # HAM warm-up: dummies timed to end as the real stream starts
            for k in range(10):
                wps = psum.tile([128, RUN], F32, tag="ps2", name="warm1",
                                bufs=2)
                nc.tensor.matmul(wps[:], r2.ones3, r2.rhs_ap(0, k * WPAD),
                                 start=True, stop=True, perf_mode=DR)
            a1v = a1p[:, GUARD:GUARD + 2 * PLANE].rearrange(
                "p (b h w i) -> p b i h w", b=BPC, i=2, h=HPAD, w=WPAD)
            xrvs = [xres[i][:].rearrange("p (b h w) -> p b h w", b=BPC, h=H)
                    for i in range(NBLK)]
            # prologue: signs woven with round-1 S1 builds (ACT order:
            # sign(b,*) then hs(b,*) so the PE's s1 stream never waits on a
            # later sample's sign)
            sg00 = None
            for b in range(BPC):
                for i in range(NBLK):
                    sg = nc.scalar.activation(a1v[:, b, i, 1:1 + H, 2:2 + W],
                                              xrvs[i][:, b], AF.Sign)
                    if sg00 is None:
                        sg00 = sg
                        # bridge dummies gated on the first sign keep the
                        # PE warm across the sign-wait jitter window
                        wg1 = None
                        for k in range(4):
                            wps = psum.tile([128, RUN], F32, tag="ps2",
                                            name="warm1b", bufs=2)
                            mm = nc.tensor.matmul(
                                wps[:], r2.ones3, r2.rhs_ap(0, (10 + k) * WPAD),
                                start=True, stop=True, perf_mode=DR)
                            if wg1 is None:
                                wg1 = mm
                        bass._add_dep_helper(wg1.ins, sg00.ins, sync=True,
                                             reason="ham bridge on sign0")
                r1.s1_build(b)
            # conv1 block 0, then block 1; one [128,4] AG for both blocks
            # (the block-0 stats chain only completes near conv1's end, so
            # an early split AG cannot beat the combined one)
            for ci in range(NCHUNK):
                r1.chunk(ci, 0)
            r1.loc_reduce(0, loc41)
            stage_half("1", loc41, 0)
            for ci in range(NCHUNK):
                r1.chunk(ci, 1)
            r1.loc_reduce(1, loc41)
            stage_half("1", loc41, 1)
            gd1 = fire4(loc41, "1")
            # keep the PE warm across the stats gap
            for k in range(16):
                wps = psum.tile([128, RUN], F32, tag="ps2", name="warm2",
                                bufs=2)
                nc.tensor.matmul(wps[:], r1.ones3, r1.rhs_ap(0, k * WPAD),
                                 start=True, stop=True, perf_mode=DR)
            wgate = None
            for k in range(26):
                wps = psum.tile([128, RUN], F32, tag="ps2", name="warm2b",
                                bufs=2)
                mm = nc.tensor.matmul(wps[:], r1.ones3,
                                      r1.rhs_ap(1, (k % 14) * WPAD),
                                      start=True, stop=True, perf_mode=DR)
                if wgate is None:
                    wgate = mm
            bass._add_dep_helper(wgate.ins, gd1.ins, sync=True,
                                 reason="ham gate on AG1b result")
            # block-0 apply runs as soon as its (earlier) AG lands, filling
            # the block-1 AG window; then per-sample block-1 apply + sign
            # feeds conv2 block 0
            r1.fin2(tot4s["1"][:])
            # sign-feed first: the conv2 stream only needs u->t->sign per
            # (sample, block); everything else (s1add, evacs, x1) can lag
            for b in range(BPC):
                r1.apply_span1(b, 0, 'dve', 'dve')
                r1.apply_span1(b, 1, 'dve', 'dve')
            for b in range(BPC):
                r2.s1_build(b)
                r2.chunk(2 * b, 0)
                r2.chunk(2 * b + 1, 0)
                r1.x1_stt(b, 0)
            r2.loc_reduce(0, loc42)
            stage_half("2", loc42, 0)
            # conv2 block 1, weaving x1(block 1) into the stream's gaps.
            # One combined [128,4] AG at the end: a split AG's first half
            # cannot fire early enough (its trigger chain completes ~13us
            # after block 0's chunks) and the second then queues behind it.
            r2.chunk(0, 1)
            r2.chunk(1, 1)
            r1.x1_stt(0, 1)
            r2.chunk(2, 1)
            r2.chunk(3, 1)
            r1.x1_stt(1, 1)
            r2.chunk(4, 1)
            r2.chunk(5, 1)
            r1.x1_stt(2, 1)
            r2.chunk(6, 1)
            r2.chunk(7, 1)
            r1.x1_stt(3, 1)
            r2.loc_reduce(1, loc42)
            stage_half("2", loc42, 1)
            gd2 = fire4(loc42, "2")
            # tail: affine on ACT, residual-add on DVE, maxout alternating
            # gp/DVE, DMA on both spare queues
            r2.fin2(tot4s["2"][:])
            r2.apply_span2(0, 0, tq=nc.vector, oq=nc.sync)
            r2.apply_span2(0, 1, tq=nc.vector, oq=nc.scalar)
            r2.apply_span2(1, 0, tq=nc.vector, oq=nc.sync)
            r2.apply_span2(1, 1, tq=nc.vector, oq=nc.scalar)

    nc.compile()
    return nc


def _prep_weight(w):
    """(O,I,3,3) fp32 -> sign lhsT (9, iblk, 128, 256) fp8 (+-1, exact),
    plus per-output-channel alpha, beta (float64)."""
    w = w.astype(np.float64)
    beta = w.mean(axis=(1, 2, 3))
    alpha = np.sqrt(((w - beta[:, None, None, None]) ** 2)
                    .mean(axis=(1, 2, 3)))
    s = np.sign(w - beta[:, None, None, None]).astype(np.float32)
    wt = s.transpose(2, 3, 1, 0).reshape(9, C, C)   # (k9, i, o)
    wt = wt.reshape(9, NBLK, 128, C)                # (k9, iblk, i, o)
    return wt.astype(ml_dtypes.float8_e4m3), alpha, beta


def make_in_maps(inputs):
    x = np.asarray(inputs['x'], np.float32)
    aa1 = float(np.asarray(inputs['alpha_a1']).reshape(-1)[0])
    aa2 = float(np.asarray(inputs['alpha_a2']).reshape(-1)[0])
    w1t, al1, be1 = _prep_weight(np.asarray(inputs['w1'], np.float32))
    w2t, al2, be2 = _prep_weight(np.asarray(inputs['w2'], np.float32))
    prm = np.zeros((128, NPRM), np.float32)
    f1 = 1.0 / (aa1 * al1)      # z scale relative to the true conv output
    f2 = 1.0 / (aa2 * al2)
    p1 = np.asarray(inputs['pos1'], np.float64)
    n1 = np.asarray(inputs['neg1'], np.float64)
    p2 = np.asarray(inputs['pos2'], np.float64)
    n2 = np.asarray(inputs['neg2'], np.float64)
    # maxout-as-max(t, n*t) requires pos == 1 (true for this model)
    assert np.allclose(p1, 1.0) and np.allclose(p2, 1.0)
    cols = ((0, np.asarray(inputs['g1'], np.float64)),
            (2, np.asarray(inputs['b1'], np.float64)),
            (4, n1),
            (6, np.asarray(inputs['g2'], np.float64)),
            (8, np.asarray(inputs['b2'], np.float64)),
            (10, n2),
            (12, BN_EPS * f1 * f1),
            (14, BN_EPS * f2 * f2),
            (16, be1 / al1),
            (18, be2 / al2))
    for base, arr in cols:
        arr = np.broadcast_to(np.asarray(arr, np.float64), (C,))
        prm[:, base] = arr[:128]
        prm[:, base + 1] = arr[128:]
    in_maps = []
    for c in range(N_CORES):
        in_maps.append({
            'x': np.ascontiguousarray(x[c * BPC:(c + 1) * BPC]),
            'w1t': w1t, 'w2t': w2t, 'prm': prm,
        })
    return in_maps


_CACHE = {}


def kernel(**inputs):
    in_maps = make_in_maps(inputs)
    if 'run' not in _CACHE:
        nc = build()
        _CACHE['nc'] = nc
        _CACHE['run'] = _make_runner(nc)
    outs = _CACHE['run'](in_maps)
    full = np.concatenate([outs[c] for c in range(N_CORES)], axis=0)
    return full.astype(np.float32)


def _make_runner(nc):
    """Build a cached PJRT executable (same path run_bass_kernel_spmd takes
    under axon, via bass2jax) so repeat calls don't re-trace."""
    import jax
    import jax.numpy as jnp
    from jax.sharding import Mesh, PartitionSpec
    from jax.experimental.shard_map import shard_map
    from concourse import bass2jax

    bass2jax.install_neuronx_cc_hook()
    partition_name = (nc.partition_id_tensor.name
                      if nc.partition_id_tensor else None)
    in_names = []
    out_names = []
    out_avals = []
    for alloc in nc.m.functions[0].allocations:
        if not isinstance(alloc, mybir.MemoryLocationSet):
            continue
        name = alloc.memorylocations[0].name
        if alloc.kind == "ExternalInput":
            if name != partition_name:
                in_names.append(name)
        elif alloc.kind == "ExternalOutput":
            shape = tuple(alloc.tensor_shape)
            dtype = mybir.dt.np(alloc.dtype)
            out_names.append(name)
            out_avals.append(jax.core.ShapedArray(shape, dtype))
    n_params = len(in_names)
    all_names = in_names + out_names
    if partition_name is not None:
        all_names = all_names + [partition_name]

    def _body(*args):
        operands = list(args)
        if partition_name is not None:
            operands.append(bass2jax.partition_id_tensor())
        outs = bass2jax._bass_exec_p.bind(
            *operands,
            out_avals=tuple(out_avals),
            in_names=tuple(all_names),
            out_names=tuple(out_names),
            lowering_input_output_aliases=(),
            sim_require_finite=True,
            sim_require_nnan=True,
            nc=nc,
        )
        return tuple(outs)

    devices = jax.devices()[:N_CORES]
    mesh = Mesh(np.asarray(devices), ("core",))
    n_outs = len(out_names)
    sharded = jax.jit(
        shard_map(_body, mesh=mesh,
                  in_specs=(PartitionSpec("core"),) * (n_params + n_outs),
                  out_specs=(PartitionSpec("core"),) * n_outs,
                  check_rep=False),
        donate_argnums=tuple(range(n_params, n_params + n_outs)),
        keep_unused=True,
    )
    sharded_nodonate = jax.jit(
        shard_map(_body, mesh=mesh,
                  in_specs=(PartitionSpec("core"),) * (n_params + n_outs),
                  out_specs=(PartitionSpec("core"),) * n_outs,
                  check_rep=False),
        keep_unused=True,
    )

    def run(in_maps):
        concat_in = [
            np.concatenate([np.asarray(in_maps[c][n]) for c in range(N_CORES)],
                           axis=0)
            for n in in_names
        ]
        concat_zeros = [
            np.zeros((N_CORES * a.shape[0], *a.shape[1:]), a.dtype)
            for a in out_avals
        ]
        out_arrs = sharded(*concat_in, *concat_zeros)
        i = out_names.index("out")
        full = np.asarray(out_arrs[i]).reshape(N_CORES, *out_avals[i].shape)
        return [full[c] for c in range(N_CORES)]

    def stage(in_maps):
        """device_put inputs once; return a dispatch closure for timing."""
        from jax.sharding import NamedSharding
        sh = NamedSharding(mesh, PartitionSpec("core"))
        concat_in = [
            jax.device_put(np.concatenate(
                [np.asarray(in_maps[c][n]) for c in range(N_CORES)], axis=0), sh)
            for n in in_names
        ]
        concat_zeros = [
            jax.device_put(
                np.zeros((N_CORES * a.shape[0], *a.shape[1:]), a.dtype), sh)
            for a in out_avals
        ]

        def dispatch():
            return sharded_nodonate(*concat_in, *concat_zeros)

        return dispatch

    run.stage = stage
    return run


# revision 38
# speedup vs baseline: 1.0608x; 1.0608x over previous
"""Trainium2 Bass kernel for a binarized (1w/1a) BasicBlock — fp8 DoubleRow.

    a1 = sign(x);  y1 = BN(conv3x3(a1, binarize(w1))) + x;  x1 = maxout(y1)
    a2 = sign(x1); y2 = BN(conv3x3(a2, binarize(w2))) + x1; out = maxout(y2)

Data-parallel over batch (4 samples/core, 8 cores); exact binary math:
activations are +-1 (fp8e4, exact), weights are sign(+-1) fp8; each conv is
9 DoubleRow matmuls per (chunk, cout-block), contracting all 256 input
channels at once over contiguous padded-row runs (pad columns land in
unused psum columns).  The two channel-block planes are byte-interleaved
per pixel so each DoubleRow rhs column is one 2-byte fetch (split planes
measured ~1.9 cycles/column; interleaved targets ~1).  conv_true = alpha_a*alpha[o]*(BB + q[o]*S1) with
q = beta/alpha; S1 (3x3 box of the channel sum) comes from 3 more DoubleRow
ones-matmuls (folding the kh taps) + 2 shifted adds.  The per-channel scale
folds into BN exactly by scaling BN_EPS per channel.

Maxout exploits pos==1: out = max(t, neg*t) — one DVE/gp STT, no sign/coef.

Batch-stat BN: one AllGather of (sum, sumsq) [128,4] per round, with the
cross-rank add done locally on DVE.  The ncfw collective stack costs
~30us of cold init (self-started from the NRT preamble ~11us in) plus
~25us on the first payload op of every execution, so a warmup AllGather
is triggered first-thing to absorb both under conv1; the two real AGs
then run at the ~5-10us steady-state floor.  Warm-up and gated dummy
matmuls bridge the PE across the stats gaps so HAM keeps the 2.4 GHz
clock.  (A GpSimd remote_dma_broadcast XOR-slot exchange would cut the
stats latency to ~3us and drop ncfw entirely, but the Tile scheduler's
single-core simulation cannot model cross-core semaphore arrivals and
deadlocks on the required wait_ge — revisit if that gains support.)
"""

import numpy as np
import ml_dtypes

import concourse.bass as bass
import concourse.bacc as bacc
import concourse.mybir as mybir
import concourse.tile as tile

N_CORES = 8
B, C, H, W = 32, 256, 28, 28
BPC = B // N_CORES            # samples per core
NBLK = 2                      # channel blocks of 128
HPAD, WPAD = 30, 32           # padded image in SBUF
PIX = H * W                   # 784
PPIX = HPAD * WPAD            # 960
NCHUNK = 2 * BPC              # 8 chunks of (sample, half-image)
HHALF = H // 2                # 14
CHUNK = HHALF * W             # 392 dense output elems per chunk
RUN = HHALF * WPAD            # 448: rhs run length / psum width per chunk
BN_EPS = 1e-5
NPRM = 20
GUARD = 32                    # fp8 guard elems around merged activation tile
PLANE = BPC * PPIX            # 3840 elems per channel-block plane
SPAN = 4 * CHUNK              # 1568-wide apply spans (2 samples)
CNT = float(NCHUNK * CHUNK * N_CORES)   # 25088 elems per channel for BN
F32 = mybir.dt.float32
BF16 = mybir.dt.bfloat16
FP8 = mybir.dt.float8e4
AF = mybir.ActivationFunctionType
ALU = mybir.AluOpType
DR = mybir.MatmulPerfMode.DoubleRow
RG = [list(range(N_CORES))]


class Rnd:
    """One conv+BN+residual+maxout round's issue helpers."""

    def __init__(self, nc, tc, pools, rnd, apad, wsb, cv, prm, pcol, onesb,
                 out_d=None, a2pad=None, xres=None, x1b=None):
        self.__dict__.update(locals())
        sbuf, psum, sc, dram = pools
        self.sc, self.psum, self.sbuf, self.dram = sc, psum, sbuf, dram
        self.sums = [sbuf.tile([128, NCHUNK], F32, name=f"sums{rnd}_{i}")
                     for i in range(NBLK)]
        self.sumsqs = [sbuf.tile([128, NCHUNK], F32, name=f"sumsqs{rnd}_{i}")
                       for i in range(NBLK)]
        # sample-major activation layout (b, i, h, w): the rhs AP's
        # contiguous dependency footprint stays inside one sample
        self.t448 = apad[:, 0:2 * PPIX].rearrange(
            "p (n i) -> p i n", i=2)[:, :, 0:RUN]
        self.wv = wsb[:].rearrange("p (k i o) -> p k i o", k=9, i=NBLK)
        self.ones3 = onesb[:].rearrange("p (i o) -> p i o", i=2)
        self.s1s = {}
        self.ss = {}
        self.tots = {}
        self.gst_dmas = {}
        self.tts = {}
        if a2pad is not None:
            self.a2v = a2pad[:, GUARD:GUARD + 2 * PLANE].rearrange(
                "p (b h w i) -> p b i h w", b=BPC, i=2, h=HPAD, w=WPAD)
        if out_d is not None:
            self.ov = out_d[:].rearrange("(bp b2) c h w -> c bp b2 (h w)",
                                         bp=2)

    def rhs_ap(self, b, off):
        return bass.AP(self.t448.tensor, GUARD + b * 2 * PPIX + 2 * off,
                       self.t448.ap)

    def s1_build(self, b):
        """S1(b) = 3x3 box of the channel sum: 3 kh-folded ones-matmuls per
        half (PE) -> hs evac (ACT) -> w-taps on gp + DVE."""
        nc, sc, psum, rnd = self.nc, self.sc, self.psum, self.rnd
        hs = sc.tile([128, 2 * RUN], F32, tag="hs", name="hs", bufs=2)
        for half in range(2):
            h0 = half * HHALF
            ps2 = psum.tile([128, RUN], F32, tag="ps2", name=f"ps2_{rnd}",
                            bufs=2)
            for kh in range(3):
                nc.tensor.matmul(ps2[:], self.ones3,
                                 self.rhs_ap(b, (h0 + kh) * WPAD),
                                 start=(kh == 0), stop=(kh == 2),
                                 perf_mode=DR)
            nc.scalar.copy(hs[:, half * RUN:half * RUN + RUN], ps2[:])
        hsv = hs[:].rearrange("p (h w) -> p h w", h=H)
        w3 = sc.tile([128, H * W], F32, tag="w3", name="w3", bufs=2)
        w3v = w3[:].rearrange("p (h w) -> p h w", h=H)
        # always DVE: GpSimd tensor ops contend with the DVE for the one
        # shared SBUF port (measured 680ns DVE TS ops degrade to 2.2us
        # whenever a gp tensor_add runs concurrently)
        nc.vector.tensor_add(w3v, hsv[:, :, 1:1 + W], hsv[:, :, 2:2 + W])
        s1 = sc.tile([128, H * W], F32, tag="s1", name="s1", bufs=BPC)
        s1v = s1[:].rearrange("p (h w) -> p h w", h=H)
        nc.vector.tensor_add(s1v, w3v, hsv[:, :, 3:3 + W])
        self.s1s[b] = s1

    def chunk(self, ci, oblk):
        """9 DoubleRow matmuls + evac: cv = q*S1 + BB (DVE STT, accumulates
        sum) and a Square pass on ScalarE (accumulates sumsq)."""
        nc, rnd = self.nc, self.rnd
        b, h0 = ci // 2, (ci % 2) * HHALF
        ps = self.psum.tile([128, RUN], F32, tag="ps", name=f"ps{rnd}",
                            bufs=6)
        for k9 in range(9):
            kh, kw = k9 // 3, k9 % 3
            nc.tensor.matmul(
                ps[:], self.wv[:, k9, :, oblk * 128:(oblk + 1) * 128],
                self.rhs_ap(b, (h0 + kh) * WPAD + kw - 1),
                start=(k9 == 0), stop=(k9 == 8), perf_mode=DR)
        psv = ps[:].rearrange("p (h w) -> p h w", h=HHALF)[:, :, 2:2 + W]
        s1v = self.s1s[b][:].rearrange("p (h w) -> p h w", h=H)[
            :, (ci % 2) * HHALF:(ci % 2) * HHALF + HHALF, :]
        cvc = self.cv[oblk][:, ci * CHUNK:(ci + 1) * CHUNK]
        nc.vector.scalar_tensor_tensor(
            cvc.rearrange("p (h w) -> p h w", h=HHALF), s1v,
            self.prm[:, self.pcol['q'] + oblk:self.pcol['q'] + oblk + 1],
            psv, op0=ALU.mult, op1=ALU.add,
            accum_out=self.sums[oblk][:, ci:ci + 1])
        sqj = self.sc.tile([128, CHUNK], F32, tag="sqj", name="sqj", bufs=2)
        nc.scalar.activation(
            sqj[:], cvc, AF.Square,
            accum_out=self.sumsqs[oblk][:, ci:ci + 1])

    def loc_reduce(self, oblk, loc4):
        """Reduce this block's per-chunk stats into loc4 cols [2b, 2b+2);
        returns the two reduce instructions (deps for the trigger)."""
        nc = self.nc
        r0 = nc.vector.reduce_sum(loc4[:, 2 * oblk:2 * oblk + 1],
                                  self.sums[oblk][:],
                                  axis=mybir.AxisListType.X)
        r1 = nc.vector.reduce_sum(loc4[:, 2 * oblk + 1:2 * oblk + 2],
                                  self.sumsqs[oblk][:],
                                  axis=mybir.AxisListType.X)
        return [r0, r1]

    def fin2(self, tot4):
        """Global (sum, sumsq) -> BN scale/shift for BOTH blocks in one
        [128,2]-wide op chain (halves the fin's cross-engine ping-pong)."""
        nc, rnd, prm, pcol = self.nc, self.rnd, self.prm, self.pcol
        sbuf = self.sbuf
        tv = tot4.rearrange("p (b j) -> p j b", b=2)
        mm = sbuf.tile([128, 2], F32, name=f"mm{rnd}")
        mq = sbuf.tile([128, 2], F32, name=f"mq{rnd}")
        var = sbuf.tile([128, 2], F32, name=f"var{rnd}")
        sd = sbuf.tile([128, 2], F32, name=f"sd{rnd}")
        inv = sbuf.tile([128, 2], F32, name=f"inv{rnd}")
        scale = sbuf.tile([128, 2], F32, name=f"scale{rnd}")
        tmp = sbuf.tile([128, 2], F32, name=f"tmp{rnd}")
        shift = sbuf.tile([128, 2], F32, name=f"shift{rnd}")
        nc.vector.tensor_scalar_mul(mm[:], tv[:, 0], 1.0 / CNT)
        nc.vector.tensor_scalar_mul(mq[:], tv[:, 1], 1.0 / CNT)
        nc.vector.tensor_mul(tmp[:], mm[:], mm[:])
        nc.vector.tensor_sub(var[:], mq[:], tmp[:])
        nc.vector.tensor_add(var[:], var[:],
                             prm[:, pcol['eps']:pcol['eps'] + 2])
        nc.scalar.activation(sd[:], var[:], AF.Sqrt)
        nc.vector.reciprocal(inv[:], sd[:])
        nc.vector.tensor_mul(scale[:], inv[:],
                             prm[:, pcol['g']:pcol['g'] + 2])
        nc.vector.tensor_mul(tmp[:], mm[:], scale[:])
        nc.vector.tensor_sub(shift[:],
                             prm[:, pcol['b']:pcol['b'] + 2], tmp[:])
        for oblk in range(NBLK):
            self.ss[oblk] = (scale[:, oblk:oblk + 1],
                             shift[:, oblk:oblk + 1])

    # ---- round-1 apply: t = scale*cv + shift + x (f32, exact signs);
    #      a2 = sign(t) (ACT); x1 = max(t, n1*t) deferred on DVE.
    def apply_span1(self, b, oblk, ueng, teng):
        nc, sc, prm, pcol = self.nc, self.sc, self.prm, self.pcol
        scale, shift = self.ss[oblk]
        lo = b * PIX
        cvs = self.cv[oblk][:, lo:lo + PIX]
        uq = nc.gpsimd if ueng == 'gp' else nc.vector
        tq = nc.gpsimd if teng == 'gp' else nc.vector
        u = sc.tile([128, PIX], F32, tag="u", name="u", bufs=3)
        uq.tensor_scalar(u[:], cvs, scale, shift,
                         op0=ALU.mult, op1=ALU.add)
        t = sc.tile([128, PIX], F32, tag="t", name="t", bufs=8)
        tq.tensor_add(t[:], u[:], self.xres[oblk][:, lo:lo + PIX])
        sg = self.a2v[:, b, oblk, 1:1 + H, 2:2 + W]
        nc.scalar.activation(sg, t[:].rearrange("p (h w) -> p h w", h=H),
                             AF.Sign)
        self.tts[(b, oblk)] = t

    def x1_stt(self, b, oblk):
        nc, prm, pcol = self.nc, self.prm, self.pcol
        t = self.tts[(b, oblk)]
        lo = b * PIX
        nc.vector.scalar_tensor_tensor(
            self.x1b[oblk][:, lo:lo + PIX], t[:],
            prm[:, pcol['n'] + oblk:pcol['n'] + oblk + 1], t[:],
            op0=ALU.mult, op1=ALU.max)

    # ---- round-2 apply: u2 = scale*cv + shift (ACT affine),
    #      t2 = u2 + x1, och = max(t2, n2*t2), DMA out.
    def apply_span2(self, sp, oblk, tq, oq, ochq=None):
        nc, sc, prm, pcol = self.nc, self.sc, self.prm, self.pcol
        scale, shift = self.ss[oblk]
        lo = sp * SPAN
        cvs = self.cv[oblk][:, lo:lo + SPAN]
        u2 = sc.tile([128, SPAN], BF16, tag="u2", name="u2", bufs=3)
        nc.scalar.activation(u2[:], cvs, AF.Identity,
                             bias=shift, scale=scale)
        t2 = sc.tile([128, SPAN], BF16, tag="t2", name="t2", bufs=3)
        tq.tensor_add(t2[:], u2[:], self.x1b[oblk][:, lo:lo + SPAN])
        nt = sc.tile([128, SPAN], BF16, tag="nt", name="nt", bufs=3)
        nc.vector.tensor_scalar_mul(
            nt[:], t2[:], prm[:, pcol['n'] + oblk:pcol['n'] + oblk + 1])
        och = sc.tile([128, SPAN], BF16, tag="och", name="och", bufs=3)
        nc.vector.tensor_max(och[:], t2[:], nt[:])
        oq.dma_start(self.ov[oblk * 128:oblk * 128 + 128, sp],
                     och[:].rearrange("p (b2 hw) -> p b2 hw", b2=2))


def build():
    nc = bacc.Bacc("TRN2", target_bir_lowering=False, debug=False,
                   enable_asserts=True, num_devices=N_CORES)
    x_d = nc.dram_tensor("x", [BPC, C, H, W], F32, kind="ExternalInput")
    w1_d = nc.dram_tensor("w1t", [9, NBLK, 128, 256], FP8,
                          kind="ExternalInput")
    w2_d = nc.dram_tensor("w2t", [9, NBLK, 128, 256], FP8,
                          kind="ExternalInput")
    prm_d = nc.dram_tensor("prm", [128, NPRM], F32, kind="ExternalInput")
    out_d = nc.dram_tensor("out", [BPC, C, H, W], BF16,
                           kind="ExternalOutput")

    with tile.TileContext(nc) as tc:
        with (
            tc.tile_pool(name="sbuf", bufs=1) as sbuf,
            tc.tile_pool(name="psum", bufs=6, space="PSUM") as psum,
            tc.tile_pool(name="sc", bufs=2) as sc,
            tc.tile_pool(name="dram", bufs=1, space="DRAM") as dram,
        ):
            pools = (sbuf, psum, sc, dram)
            w1sb = sbuf.tile([128, 9 * NBLK * 256], FP8, name="w1sb")
            w2sb = sbuf.tile([128, 9 * NBLK * 256], FP8, name="w2sb")
            prm = sbuf.tile([128, NPRM], F32, name="prm")
            onesb = sbuf.tile([128, 256], FP8, name="onesb")
            nc.vector.memset(onesb[:], 1.0)
            xres = [sbuf.tile([128, BPC * PIX], F32, name=f"xres{i}")
                    for i in range(NBLK)]
            x1b = [sbuf.tile([128, BPC * PIX], BF16, name=f"x1b{i}")
                   for i in range(NBLK)]
            a1p = sbuf.tile([128, GUARD + 2 * PLANE + GUARD], FP8, name="a1p")
            a2p = sbuf.tile([128, GUARD + 2 * PLANE + GUARD], FP8, name="a2p")
            cv = [sbuf.tile([128, BPC * PIX], F32, name=f"cv{i}")
                  for i in range(NBLK)]
            cvb = [sbuf.tile([128, BPC * PIX], BF16, name=f"cvb{i}")
                   for i in range(NBLK)]

            nc.vector.memset(a1p[:].bitcast(mybir.dt.uint32), 0)
            nc.gpsimd.memset(a2p[:].bitcast(mybir.dt.uint32), 0)
            # x planes all on one ring (sample-major: one ring fans out over
            # all 16 DMA engines at full BW; splitting rings fragments it)
            xv = x_d[:].rearrange("b c h w -> c b (h w)")
            for b in range(BPC):
                for i in range(NBLK):
                    nc.sync.dma_start(xres[i][:, b * PIX:(b + 1) * PIX],
                                      xv[i * 128:(i + 1) * 128, b])
            w1v = w1sb[:].rearrange("p (k i o) -> p k i o", k=9, i=NBLK)
            w1dv = w1_d[:].rearrange("k i p o -> p k i o")
            nc.scalar.dma_start(w1v[:, 0:3], w1dv[:, 0:3])
            nc.gpsimd.dma_start(w1v[:, 3:6], w1dv[:, 3:6])
            nc.sync.dma_start(w1v[:, 6:9], w1dv[:, 6:9])
            nc.sync.dma_start(prm[:], prm_d[:])
            nc.sync.dma_start(
                w2sb[:].rearrange("p (k i o) -> p k i o", k=9, i=NBLK),
                w2_d[:].rearrange("k i p o -> p k i o"))

            pcol1 = {'g': 0, 'b': 2, 'n': 4, 'eps': 12, 'q': 16}
            pcol2 = {'g': 6, 'b': 8, 'n': 10, 'eps': 14, 'q': 18}
            r1 = Rnd(nc, tc, pools, 1, a1p, w1sb, cv, prm, pcol1, onesb,
                     a2pad=a2p, xres=xres, x1b=x1b)
            r2 = Rnd(nc, tc, pools, 2, a2p, w2sb, cvb, prm, pcol2, onesb,
                     out_d=out_d, x1b=x1b)

            # --- stats exchange state (ncfw AllGather; the warmup AG
            # below absorbs the ~30us ncfw init + ~25us first-op cost) ---
            loc41 = sbuf.tile([128, 4], F32, name="loc41")
            loc42 = sbuf.tile([128, 4], F32, name="loc42")
            tot4s = {k: sbuf.tile([128, 4], F32, name=f"tot4_{k}")
                     for k in ("1", "2")}
            # force the gpsimd tensor lib to load in the idle prologue (its
            # lazy ~6us IRAM load otherwise stalls the first transition add)
            gwarm = sbuf.tile([128, 2], F32, name="gwarm")
            nc.vector.memset(gwarm[:], 0.0)
            nc.gpsimd.tensor_add(gwarm[:, 0:1], gwarm[:, 0:1], gwarm[:, 1:2])
            # warmup collective: absorbs the ~30us ncfw init + ~25us
            # first-payload-op cost under conv1
            wu_i = dram.tile([1, 16], F32, name="wu_i")
            wu_o = dram.tile([N_CORES, 16], F32, name="wu_o")
            with tc.high_priority():
                nc.sync.dma_start(
                    wu_i[:],
                    x_d[:].rearrange("b c h w -> (b c h) w")[0:1, 0:16])
                nc.gpsimd.collective_compute(
                    "AllGather", ALU.bypass, replica_groups=RG,
                    ins=[wu_i.opt()], outs=[wu_o.opt()])

            a_ins = {k: dram.tile([128, 4], F32, name=f"agin{k}")
                     for k in ("1", "2")}

            def stage_half(key, loc4, blk):
                nc.sync.dma_start(a_ins[key][:, 2 * blk:2 * blk + 2],
                                  loc4[:, 2 * blk:2 * blk + 2])

            def fire4(loc4, key):
                a_in = a_ins[key]
                a_out = dram.tile([N_CORES, 128, 4], F32, name=f"agout{key}")
                nc.gpsimd.collective_compute(
                    "AllGather", ALU.bypass, replica_groups=RG,
                    ins=[a_in.opt()], outs=[a_out.opt()])
                gst = sbuf.tile([128, N_CORES * 4], F32, name=f"gst{key}")
                gd = nc.sync.dma_start(
                    gst[:].rearrange("p (r j) -> p r j", r=N_CORES),
                    a_out[:].rearrange("r p j -> p r j"))
                nc.vector.reduce_sum(
                    tot4s[key][:],
                    gst[:].rearrange("p (r j) -> p j r", r=N_CORES),
                    axis=mybir.AxisListType.X)
                return gd

            def fire(loc2, key):
                a_in = dram.tile([128, 2], F32, name=f"agin{key}")
                a_out = dram.tile([N_CORES, 128, 2], F32, name=f"agout{key}")
                nc.sync.dma_start(a_in[:], loc2)
                nc.gpsimd.collective_compute(
                    "AllGather", ALU.bypass, replica_groups=RG,
                    ins=[a_in.opt()], outs=[a_out.opt()])
                gst = sbuf.tile([128, N_CORES * 2], F32, name=f"gst{key}")
                gd = nc.sync.dma_start(
                    gst[:].rearrange("p (r j) -> p r j", r=N_CORES),
                    a_out[:].rearrange("r p j -> p r j"))
                nc.vector.reduce_sum(
                    tots[key][:],
                    gst[:].rearrange("p (r j) -> p j r", r=N_CORES),
                    axis=mybir.AxisListType.X)
                return gd

            # HAM warm-up: dummies timed to end as the real stream starts
            for k in range(10):
                wps = psum.tile([128, RUN], F32, tag="ps2", name="warm1",
                                bufs=2)
                nc.tensor.matmul(wps[:], r2.ones3, r2.rhs_ap(0, k * WPAD),
                                 start=True, stop=True, perf_mode=DR)
            a1v = a1p[:, GUARD:GUARD + 2 * PLANE].rearrange(
                "p (b h w i) -> p b i h w", b=BPC, i=2, h=HPAD, w=WPAD)
            xrvs = [xres[i][:].rearrange("p (b h w) -> p b h w", b=BPC, h=H)
                    for i in range(NBLK)]
            # prologue: signs woven with round-1 S1 builds (ACT order:
            # sign(b,*) then hs(b,*) so the PE's s1 stream never waits on a
            # later sample's sign)
            sg00 = None
            for b in range(BPC):
                for i in range(NBLK):
                    sg = nc.scalar.activation(a1v[:, b, i, 1:1 + H, 2:2 + W],
                                              xrvs[i][:, b], AF.Sign)
                    if sg00 is None:
                        sg00 = sg
                        # bridge dummies gated on the first sign keep the
                        # PE warm across the sign-wait jitter window
                        wg1 = None
                        for k in range(4):
                            wps = psum.tile([128, RUN], F32, tag="ps2",
                                            name="warm1b", bufs=2)
                            mm = nc.tensor.matmul(
                                wps[:], r2.ones3, r2.rhs_ap(0, (10 + k) * WPAD),
                                start=True, stop=True, perf_mode=DR)
                            if wg1 is None:
                                wg1 = mm
                        bass._add_dep_helper(wg1.ins, sg00.ins, sync=True,
                                             reason="ham bridge on sign0")
                r1.s1_build(b)
            # conv1 block 0, then block 1; one [128,4] AG for both blocks
            # (the block-0 stats chain only completes near conv1's end, so
            # an early split AG cannot beat the combined one)
            for ci in range(NCHUNK):
                r1.chunk(ci, 0)
            r1.loc_reduce(0, loc41)
            stage_half("1", loc41, 0)
            for ci in range(NCHUNK):
                r1.chunk(ci, 1)
            r1.loc_reduce(1, loc41)
            stage_half("1", loc41, 1)
            gd1 = fire4(loc41, "1")
            # keep the PE warm across the stats gap
            for k in range(16):
                wps = psum.tile([128, RUN], F32, tag="ps2", name="warm2",
                                bufs=2)
                nc.tensor.matmul(wps[:], r1.ones3, r1.rhs_ap(0, k * WPAD),
                                 start=True, stop=True, perf_mode=DR)
            wgate = None
            for k in range(10):
                wps = psum.tile([128, RUN], F32, tag="ps2", name="warm2b",
                                bufs=2)
                mm = nc.tensor.matmul(wps[:], r1.ones3,
                                      r1.rhs_ap(1, k * WPAD),
                                      start=True, stop=True, perf_mode=DR)
                if wgate is None:
                    wgate = mm
            bass._add_dep_helper(wgate.ins, gd1.ins, sync=True,
                                 reason="ham gate on AG1b result")
            # block-0 apply runs as soon as its (earlier) AG lands, filling
            # the block-1 AG window; then per-sample block-1 apply + sign
            # feeds conv2 block 0
            r1.fin2(tot4s["1"][:])
            # sign-feed first: the conv2 stream only needs u->t->sign per
            # (sample, block); everything else (s1add, evacs, x1) can lag
            for b in range(BPC):
                r1.apply_span1(b, 0, 'dve', 'dve')
                r1.apply_span1(b, 1, 'dve', 'dve')
            for b in range(BPC):
                r2.s1_build(b)
                r2.chunk(2 * b, 0)
                r2.chunk(2 * b + 1, 0)
                r1.x1_stt(b, 0)
            r2.loc_reduce(0, loc42)
            stage_half("2", loc42, 0)
            # conv2 block 1, weaving x1(block 1) into the stream's gaps.
            # One combined [128,4] AG at the end: a split AG's first half
            # cannot fire early enough (its trigger chain completes ~13us
            # after block 0's chunks) and the second then queues behind it.
            r2.chunk(0, 1)
            r2.chunk(1, 1)
            r1.x1_stt(0, 1)
            r2.chunk(2, 1)
            r2.chunk(3, 1)
            r1.x1_stt(1, 1)
            r2.chunk(4, 1)
            r2.chunk(5, 1)
            r1.x1_stt(2, 1)
            r2.chunk(6, 1)
            r2.chunk(7, 1)
            r1.x1_stt(3, 1)
            r2.loc_reduce(1, loc42)
            stage_half("2", loc42, 1)
            gd2 = fire4(loc42, "2")
            # tail: affine on ACT, residual-add on DVE, maxout alternating
            # gp/DVE, DMA on both spare queues
            r2.fin2(tot4s["2"][:])
            r2.apply_span2(0, 0, tq=nc.vector, oq=nc.sync)
            r2.apply_span2(0, 1, tq=nc.vector, oq=nc.scalar)
            r2.apply_span2(1, 0, tq=nc.vector, oq=nc.sync)
            r2.apply_span2(1, 1, tq=nc.vector, oq=nc.scalar)

    nc.compile()
    return nc


def _prep_weight(w):
    """(O,I,3,3) fp32 -> sign lhsT (9, iblk, 128, 256) fp8 (+-1, exact),
    plus per-output-channel alpha, beta (float64)."""
    w = w.astype(np.float64)
    beta = w.mean(axis=(1, 2, 3))
    alpha = np.sqrt(((w - beta[:, None, None, None]) ** 2)
                    .mean(axis=(1, 2, 3)))
    s = np.sign(w - beta[:, None, None, None]).astype(np.float32)
    wt = s.transpose(2, 3, 1, 0).reshape(9, C, C)   # (k9, i, o)
    wt = wt.reshape(9, NBLK, 128, C)                # (k9, iblk, i, o)
    return wt.astype(ml_dtypes.float8_e4m3), alpha, beta


def make_in_maps(inputs):
    x = np.asarray(inputs['x'], np.float32)
    aa1 = float(np.asarray(inputs['alpha_a1']).reshape(-1)[0])
    aa2 = float(np.asarray(inputs['alpha_a2']).reshape(-1)[0])
    w1t, al1, be1 = _prep_weight(np.asarray(inputs['w1'], np.float32))
    w2t, al2, be2 = _prep_weight(np.asarray(inputs['w2'], np.float32))
    prm = np.zeros((128, NPRM), np.float32)
    f1 = 1.0 / (aa1 * al1)      # z scale relative to the true conv output
    f2 = 1.0 / (aa2 * al2)
    p1 = np.asarray(inputs['pos1'], np.float64)
    n1 = np.asarray(inputs['neg1'], np.float64)
    p2 = np.asarray(inputs['pos2'], np.float64)
    n2 = np.asarray(inputs['neg2'], np.float64)
    # maxout-as-max(t, n*t) requires pos == 1 (true for this model)
    assert np.allclose(p1, 1.0) and np.allclose(p2, 1.0)
    cols = ((0, np.asarray(inputs['g1'], np.float64)),
            (2, np.asarray(inputs['b1'], np.float64)),
            (4, n1),
            (6, np.asarray(inputs['g2'], np.float64)),
            (8, np.asarray(inputs['b2'], np.float64)),
            (10, n2),
            (12, BN_EPS * f1 * f1),
            (14, BN_EPS * f2 * f2),
            (16, be1 / al1),
            (18, be2 / al2))
    for base, arr in cols:
        arr = np.broadcast_to(np.asarray(arr, np.float64), (C,))
        prm[:, base] = arr[:128]
        prm[:, base + 1] = arr[128:]
    in_maps = []
    for c in range(N_CORES):
        in_maps.append({
            'x': np.ascontiguousarray(x[c * BPC:(c + 1) * BPC]),
            'w1t': w1t, 'w2t': w2t, 'prm': prm,
        })
    return in_maps


_CACHE = {}


def kernel(**inputs):
    in_maps = make_in_maps(inputs)
    if 'run' not in _CACHE:
        nc = build()
        _CACHE['nc'] = nc
        _CACHE['run'] = _make_runner(nc)
    outs = _CACHE['run'](in_maps)
    full = np.concatenate([outs[c] for c in range(N_CORES)], axis=0)
    return full.astype(np.float32)


def _make_runner(nc):
    """Build a cached PJRT executable (same path run_bass_kernel_spmd takes
    under axon, via bass2jax) so repeat calls don't re-trace."""
    import jax
    import jax.numpy as jnp
    from jax.sharding import Mesh, PartitionSpec
    from jax.experimental.shard_map import shard_map
    from concourse import bass2jax

    bass2jax.install_neuronx_cc_hook()
    partition_name = (nc.partition_id_tensor.name
                      if nc.partition_id_tensor else None)
    in_names = []
    out_names = []
    out_avals = []
    for alloc in nc.m.functions[0].allocations:
        if not isinstance(alloc, mybir.MemoryLocationSet):
            continue
        name = alloc.memorylocations[0].name
        if alloc.kind == "ExternalInput":
            if name != partition_name:
                in_names.append(name)
        elif alloc.kind == "ExternalOutput":
            shape = tuple(alloc.tensor_shape)
            dtype = mybir.dt.np(alloc.dtype)
            out_names.append(name)
            out_avals.append(jax.core.ShapedArray(shape, dtype))
    n_params = len(in_names)
    all_names = in_names + out_names
    if partition_name is not None:
        all_names = all_names + [partition_name]

    def _body(*args):
        operands = list(args)
        if partition_name is not None:
            operands.append(bass2jax.partition_id_tensor())
        outs = bass2jax._bass_exec_p.bind(
            *operands,
            out_avals=tuple(out_avals),
            in_names=tuple(all_names),
            out_names=tuple(out_names),
            lowering_input_output_aliases=(),
            sim_require_finite=True,
            sim_require_nnan=True,
            nc=nc,
        )
        return tuple(outs)

    devices = jax.devices()[:N_CORES]
    mesh = Mesh(np.asarray(devices), ("core",))
    n_outs = len(out_names)
    sharded = jax.jit(
        shard_map(_body, mesh=mesh,
                  in_specs=(PartitionSpec("core"),) * (n_params + n_outs),
                  out_specs=(PartitionSpec("core"),) * n_outs,
                  check_rep=False),
        donate_argnums=tuple(range(n_params, n_params + n_outs)),
        keep_unused=True,
    )
    sharded_nodonate = jax.jit(
        shard_map(_body, mesh=mesh,
                  in_specs=(PartitionSpec("core"),) * (n_params + n_outs),
                  out_specs=(PartitionSpec("core"),) * n_outs,
                  check_rep=False),
        keep_unused=True,
    )

    def run(in_maps):
        concat_in = [
            np.concatenate([np.asarray(in_maps[c][n]) for c in range(N_CORES)],
                           axis=0)
            for n in in_names
        ]
        concat_zeros = [
            np.zeros((N_CORES * a.shape[0], *a.shape[1:]), a.dtype)
            for a in out_avals
        ]
        out_arrs = sharded(*concat_in, *concat_zeros)
        i = out_names.index("out")
        full = np.asarray(out_arrs[i]).reshape(N_CORES, *out_avals[i].shape)
        return [full[c] for c in range(N_CORES)]

    def stage(in_maps):
        """device_put inputs once; return a dispatch closure for timing."""
        from jax.sharding import NamedSharding
        sh = NamedSharding(mesh, PartitionSpec("core"))
        concat_in = [
            jax.device_put(np.concatenate(
                [np.asarray(in_maps[c][n]) for c in range(N_CORES)], axis=0), sh)
            for n in in_names
        ]
        concat_zeros = [
            jax.device_put(
                np.zeros((N_CORES * a.shape[0], *a.shape[1:]), a.dtype), sh)
            for a in out_avals
        ]

        def dispatch():
            return sharded_nodonate(*concat_in, *concat_zeros)

        return dispatch

    run.stage = stage
    return run
